# revision 4
# baseline (speedup 1.0000x reference)
"""Trainium2 Bass kernel for a 2-layer stacked GRU (Keras reset_after=True)
with sequence masking.

Problem: x [64, 512, 512], 2x GRU(512) with mask; returns (output [B,T,U],
h0_final [B,U], h1_final [B,U]).

Strategy (data-parallel over batch, 8 cores x 8 sequences each):
  Phase A: GI0 = x @ W0 (+ input bias) batched over all T  -> big PE matmul
  Phase B: layer-0 recurrence over T steps (gates-major layout [128, tiles, b])
  Phase C: GI1 = h0_seq @ W1 batched over all T
  Phase D: layer-1 recurrence; the masked h1 state sequence IS the reference
           output (out_t and h1_t follow identical select recurrences).

Per-step recurrence math (w = sigmoid(-(iz+hz)) = 1-z):
  h_next = h + m * w * (tanh(ih + r*hh) - h)

Layouts per core (partition dim first):
  xT      [128, 4, T, 8]   bf16  (d%128, d//128, t, b)
  W/U     [128, 4, 12, 128] bf16 (k%128, k//128, gate_tile, gate%128)
  gi      [128, 12, T, 8]  bf16  (g%128, g//128, t, b)  DRAM scratch
  h/seq   [128, T, 4, 8]         (u%128, t, u//128, b)
"""

import os
import sys

for _p in ("/opt/trn_rl_repo",):
    if _p not in sys.path:
        sys.path.append(_p)

import numpy as np

import concourse.bass as bass
import concourse.tile as tile
from concourse import mybir, bacc
from concourse.bass_utils import run_bass_kernel_spmd

B, T, D, U = 64, 512, 512, 512
G = 3 * U
N_CORES = 8
BC = B // N_CORES      # 8 sequences per core
UT = U // 128          # 4 unit tiles
GT = G // 128          # 12 gate tiles
TC = 64                # time-chunk for the big matmuls (moving free = TC*BC = 512)
CH = 16                # time-chunk for the recurrences

F32 = mybir.dt.float32
BF16 = mybir.dt.float16  # "BF16" name kept; fp16 has 10 mantissa bits, same FWL speed
AF = mybir.ActivationFunctionType
OP = mybir.AluOpType

_BUILD_CACHE = {}


def _build(has_mask: bool):
    nc = bacc.Bacc("TRN2", target_bir_lowering=False, debug=False,
                   num_devices=N_CORES)

    xT_h = nc.dram_tensor("xT", (128, UT, T, BC), BF16, kind="ExternalInput")
    w0_h = nc.dram_tensor("w0t", (128, UT, GT, 128), BF16, kind="ExternalInput")
    u0_h = nc.dram_tensor("u0t", (128, UT, GT, 128), BF16, kind="ExternalInput")
    w1_h = nc.dram_tensor("w1t", (128, UT, GT, 128), BF16, kind="ExternalInput")
    u1_h = nc.dram_tensor("u1t", (128, UT, GT, 128), BF16, kind="ExternalInput")
    msk_h = None
    if has_mask:
        msk_h = nc.dram_tensor("maskr", (T, UT, BC), F32, kind="ExternalInput")

    gi0_h = nc.dram_tensor("gi0", (128, GT, T, BC), BF16, kind="Internal")
    gi1_h = nc.dram_tensor("gi1", (128, GT, T, BC), BF16, kind="Internal")
    h0s_h = nc.dram_tensor("h0seq", (128, T, UT, BC), BF16, kind="Internal")

    out_h = nc.dram_tensor("h1seq", (128, T, UT, BC), F32, kind="ExternalOutput")
    h0f_h = nc.dram_tensor("h0f", (128, UT, BC), F32, kind="ExternalOutput")
    h1f_h = nc.dram_tensor("h1f", (128, UT, BC), F32, kind="ExternalOutput")

    from contextlib import ExitStack

    with tile.TileContext(nc) as tc, ExitStack() as ctx:
        wpool = ctx.enter_context(tc.tile_pool(name="weights", bufs=1))
        xpool = ctx.enter_context(tc.tile_pool(name="rhschunk", bufs=2))
        gpool = ctx.enter_context(tc.tile_pool(name="gichunk", bufs=2))
        opool = ctx.enter_context(tc.tile_pool(name="giout", bufs=4))
        hpool = ctx.enter_context(tc.tile_pool(name="hseq", bufs=2))
        mpool = ctx.enter_context(tc.tile_pool(name="mask", bufs=2))
        spool = ctx.enter_context(tc.tile_pool(name="state", bufs=1))
        epool = ctx.enter_context(tc.tile_pool(name="ew", bufs=3))
        pspool = ctx.enter_context(tc.tile_pool(name="ps", bufs=4, space="PSUM"))

        def load_w(h):
            t = wpool.tile([128, UT, GT, 128], BF16, tag=h.name)
            nc.sync.dma_start(out=t[:], in_=h.ap())
            return t

        w0sb = load_w(w0_h)
        u0sb = load_w(u0_h)
        w1sb = load_w(w1_h)
        u1sb = load_w(u1_h)

        def big_matmul(wsb, load_rhs_ut_view, gi_h):
            """gi[:, gt, t, b] = sum_k w[k, g] * rhs[k, t, b], t-chunked."""
            for ck in range(T // TC):
                rhs_ut = load_rhs_ut_view(ck)
                for gt in range(GT):
                    ps = pspool.tile([128, TC, BC], F32, tag="mm")
                    for ut in range(UT):
                        nc.tensor.matmul(
                            ps[:], wsb[:, ut, gt, :], rhs_ut(ut),
                            start=(ut == 0), stop=(ut == UT - 1),
                        )
                    ob = opool.tile([128, TC, BC], BF16, tag="giout")
                    nc.vector.tensor_copy(out=ob[:], in_=ps[:])
                    nc.sync.dma_start(
                        out=gi_h.ap()[:, gt, ck * TC:(ck + 1) * TC, :], in_=ob[:]
                    )

        def load_x_chunk(ck):
            xc = xpool.tile([128, UT, TC, BC], BF16, tag="rhs")
            nc.sync.dma_start(
                out=xc[:], in_=xT_h.ap()[:, :, ck * TC:(ck + 1) * TC, :]
            )
            return lambda ut: xc[:, ut, :, :]

        def load_h0_chunk(ck):
            hc = xpool.tile([128, TC, UT, BC], BF16, tag="rhs")
            nc.sync.dma_start(
                out=hc[:], in_=h0s_h.ap()[:, ck * TC:(ck + 1) * TC, :, :]
            )
            return lambda ut: hc[:, :, ut, :]

        def recurrence(usb, gi_h, seq_bf_h, seq_f32_h, hf_h):
            hprev_f = spool.tile([128, UT, BC], F32, tag="h0init")
            hprev_b = spool.tile([128, UT, BC], BF16, tag="h0initb")
            nc.vector.memset(hprev_f[:], 0.0)
            nc.vector.memset(hprev_b[:], 0.0)
            hprev_f = hprev_f[:]
            hprev_b = hprev_b[:]

            for c in range(T // CH):
                gic = gpool.tile([128, GT, CH, BC], BF16, tag="gic")
                nc.sync.dma_start(
                    out=gic[:], in_=gi_h.ap()[:, :, c * CH:(c + 1) * CH, :]
                )
                if has_mask:
                    mskc = mpool.tile([128, CH, UT, BC], F32, tag="mc")
                    msl = msk_h.ap()[c * CH:(c + 1) * CH]
                    nc.gpsimd.dma_start(
                        out=mskc[:],
                        in_=bass.AP(tensor=msl.tensor, offset=msl.offset,
                                    ap=[[0, 128]] + list(msl.ap)),
                    )
                hfb = hpool.tile([128, CH, UT, BC], F32, tag="hf")
                hbb = hpool.tile([128, CH, UT, BC], BF16, tag="hb")

                for tt in range(CH):
                    ps = pspool.tile([128, GT, BC], F32, tag="mm")
                    for gt in range(GT):
                        for ut in range(UT):
                            nc.tensor.matmul(
                                ps[:, gt, :], usb[:, ut, gt, :],
                                hprev_b[:, ut, :],
                                start=(ut == 0), stop=(ut == UT - 1),
                            )
                    # EW: gates-major [128, *, BC]
                    t1 = epool.tile([128, 2 * UT, BC], F32, tag="t1")
                    nc.vector.tensor_add(t1[:], ps[:, 0:2 * UT, :],
                                         gic[:, 0:2 * UT, tt, :])
                    w_ = epool.tile([128, UT, BC], F32, tag="w")
                    nc.scalar.activation(out=w_[:], in_=t1[:, 0:UT, :],
                                         func=AF.Sigmoid, scale=-1.0)
                    r_ = epool.tile([128, UT, BC], F32, tag="r")
                    nc.scalar.activation(out=r_[:], in_=t1[:, UT:2 * UT, :],
                                         func=AF.Sigmoid)
                    hr = epool.tile([128, UT, BC], F32, tag="hr")
                    nc.vector.tensor_mul(hr[:], r_[:], ps[:, 2 * UT:3 * UT, :])
                    cp = epool.tile([128, UT, BC], F32, tag="cp")
                    nc.vector.tensor_add(cp[:], hr[:], gic[:, 2 * UT:3 * UT, tt, :])
                    cd = epool.tile([128, UT, BC], F32, tag="cd")
                    nc.scalar.activation(out=cd[:], in_=cp[:], func=AF.Tanh)
                    if has_mask:
                        wm = epool.tile([128, UT, BC], F32, tag="wm")
                        nc.vector.tensor_mul(wm[:], w_[:], mskc[:, tt, :, :])
                    else:
                        wm = w_
                    d_ = epool.tile([128, UT, BC], F32, tag="d")
                    nc.vector.tensor_sub(d_[:], cd[:], hprev_f)
                    e_ = epool.tile([128, UT, BC], F32, tag="e")
                    nc.vector.tensor_mul(e_[:], wm[:], d_[:])
                    nc.vector.tensor_add(hfb[:, tt], hprev_f, e_[:])
                    nc.vector.tensor_copy(out=hbb[:, tt], in_=hfb[:, tt])
                    hprev_f = hfb[:, tt]
                    hprev_b = hbb[:, tt]

                if seq_bf_h is not None:
                    nc.sync.dma_start(
                        out=seq_bf_h.ap()[:, c * CH:(c + 1) * CH, :, :], in_=hbb[:]
                    )
                if seq_f32_h is not None:
                    nc.sync.dma_start(
                        out=seq_f32_h.ap()[:, c * CH:(c + 1) * CH, :, :], in_=hfb[:]
                    )
            nc.sync.dma_start(out=hf_h.ap(), in_=hprev_f)

        big_matmul(w0sb, load_x_chunk, gi0_h)
        recurrence(u0sb, gi0_h, h0s_h, None, h0f_h)
        big_matmul(w1sb, load_h0_chunk, gi1_h)
        recurrence(u1sb, gi1_h, None, out_h, h1f_h)

    nc.compile()
    return nc


def _get_nc(has_mask: bool):
    key = has_mask
    if key not in _BUILD_CACHE:
        _BUILD_CACHE[key] = _build(has_mask)
    return _BUILD_CACHE[key]


def kernel(x, mask, W0, U0, b0, W1, U1, b1):
    x = np.asarray(x, np.float32)
    mask = np.asarray(mask)
    assert x.shape == (B, T, D) and mask.shape == (B, T)

    if np.any(np.asarray(b0)) or np.any(np.asarray(b1)):
        raise NotImplementedError("nonzero GRU biases not supported")

    has_mask = not bool(mask.all())
    nc = _get_nc(has_mask)

    def pack_w(w):
        return np.ascontiguousarray(
            np.asarray(w, np.float32).reshape(UT, 128, GT, 128)
            .transpose(1, 0, 2, 3)
        ).astype(np.float16)

    w0t, u0t, w1t, u1t = pack_w(W0), pack_w(U0), pack_w(W1), pack_w(U1)

    in_maps = []
    for c in range(N_CORES):
        xc = x[c * BC:(c + 1) * BC]                      # [BC, T, D]
        xT = np.ascontiguousarray(
            xc.reshape(BC, T, UT, 128).transpose(3, 2, 1, 0)
        ).astype(np.float16)                      # [128, UT, T, BC]
        m = {"xT": xT, "w0t": w0t, "u0t": u0t, "w1t": w1t, "u1t": u1t}
        if has_mask:
            mc = mask[c * BC:(c + 1) * BC].T.astype(np.float32)   # [T, BC]
            m["maskr"] = np.ascontiguousarray(
                np.broadcast_to(mc[:, None, :], (T, UT, BC))
            )
        in_maps.append(m)

    res = run_bass_kernel_spmd(
        nc, in_maps, core_ids=list(range(N_CORES)),
        trace=bool(os.environ.get("BASS_GRU_TRACE")),
    )
    global LAST_RESULTS
    LAST_RESULTS = res

    output = np.empty((B, T, U), np.float32)
    h0f = np.empty((B, U), np.float32)
    h1f = np.empty((B, U), np.float32)
    for c in range(N_CORES):
        r = res.results[c]
        # h1seq [128, T, UT, BC] -> [BC, T, UT*128]
        output[c * BC:(c + 1) * BC] = (
            r["h1seq"].transpose(3, 1, 2, 0).reshape(BC, T, U)
        )
        h0f[c * BC:(c + 1) * BC] = r["h0f"].transpose(2, 1, 0).reshape(BC, U)
        h1f[c * BC:(c + 1) * BC] = r["h1f"].transpose(2, 1, 0).reshape(BC, U)
    return output, h0f, h1f


# revision 6
# speedup vs baseline: 1.5647x; 1.5647x over previous
"""Trainium2 Bass kernel for a 2-layer stacked GRU (Keras reset_after=True)
with sequence masking.

Problem: x [64, 512, 512], 2x GRU(512) with mask; returns (output [B,T,U],
h0_final [B,U], h1_final [B,U]).

Strategy: data-parallel over batch (8 cores x 8 sequences each). On each core
a single software pipeline over time-chunks of CH steps:

    GI0(c): W0-matmuls on x chunk c        -> gi0 SBUF ring   (PE + ACT copy)
    L0(c):  CH steps of layer-0 recurrence -> h0 chunk ring
    GI1(c): W1-matmuls on h0 chunk c       -> gi1 SBUF ring
    L1(c):  CH steps of layer-1 recurrence -> output chunk (DMA out)

L1 runs one chunk behind L0 and their per-step emissions are interleaved, so
each layer's serial elementwise chain hides under the other layer's matmul
block. GI0/GI1 matmul slices are spread between steps as extra PE filler.

The mask is folded into the z-gate pre-activation in GI (an extra K=1 matmul
adds +BIG*(1-m_t) to iz), which makes z ~= 1 (w ~= 0) on masked steps, so
h carries through unchanged - no per-step mask ops at all. The masked h1
state sequence equals the reference output exactly (out_t and h1_t follow
identical select recurrences, and layer-1's input on masked steps is never
observable in any output).

Per-step math (w = sigmoid(-(iz+hz)) = 1-z):
    h_next = h + w * (tanh(ih + r*hh) - h)

All matmul operands fp16 (10-bit mantissa; FWL-eligible like bf16), PSUM
accumulation fp32, h state carried in fp16, pre-activations assembled in f32.

Layouts per core (partition dim first):
  xT      [128, 4, T, 8]    fp16  (d%128, d//128, t, b)
  W/U     [128, 4, 12, 128] fp16  (k%128, k//128, gate_tile, gate%128)
  gi ring [128, 12, CH, 8]  fp16  (g%128, g//128, tt, b)
  h ring  [128, CH, 4, 8]   fp16  (u%128, tt, u//128, b)
"""

import os
import sys

for _p in ("/opt/trn_rl_repo",):
    if _p not in sys.path:
        sys.path.append(_p)

import numpy as np

import concourse.bass as bass
import concourse.tile as tile
from concourse import mybir, bacc
from concourse.bass_utils import run_bass_kernel_spmd

B, T, D, U = 64, 512, 512, 512
G = 3 * U
N_CORES = 8
BC = B // N_CORES      # 8 sequences per core
UT = U // 128          # 4 unit tiles
GT = G // 128          # 12 gate tiles
CH = 16                # time-chunk (pipeline granularity)
BIG = 30.0             # z-gate pre-activation offset on masked steps

F32 = mybir.dt.float32
F16 = mybir.dt.float16
AF = mybir.ActivationFunctionType
OP = mybir.AluOpType

_BUILD_CACHE = {}


def _build(has_mask: bool):
    nc = bacc.Bacc("TRN2", target_bir_lowering=False, debug=False,
                   num_devices=N_CORES)

    xT_h = nc.dram_tensor("xT", (128, UT, T, BC), F16, kind="ExternalInput")
    w0_h = nc.dram_tensor("w0t", (128, UT, GT, 128), F16, kind="ExternalInput")
    u0_h = nc.dram_tensor("u0t", (128, UT, GT, 128), F16, kind="ExternalInput")
    w1_h = nc.dram_tensor("w1t", (128, UT, GT, 128), F16, kind="ExternalInput")
    u1_h = nc.dram_tensor("u1t", (128, UT, GT, 128), F16, kind="ExternalInput")
    msk_h = None
    if has_mask:
        # (1 - mask) per (t, b), fp16
        msk_h = nc.dram_tensor("maskn", (T, BC), F16, kind="ExternalInput")

    out_h = nc.dram_tensor("h1seq", (128, T, UT, BC), F16, kind="ExternalOutput")
    h0f_h = nc.dram_tensor("h0f", (128, UT, BC), F16, kind="ExternalOutput")
    h1f_h = nc.dram_tensor("h1f", (128, UT, BC), F16, kind="ExternalOutput")

    NCH = T // CH

    from contextlib import ExitStack

    with tile.TileContext(nc) as tc, ExitStack() as ctx:
        wpool = ctx.enter_context(tc.tile_pool(name="weights", bufs=1))
        xpool = ctx.enter_context(tc.tile_pool(name="xchunk", bufs=3))
        gpool = ctx.enter_context(tc.tile_pool(name="girings", bufs=3))
        hpool = ctx.enter_context(tc.tile_pool(name="hrings", bufs=3))
        mpool = ctx.enter_context(tc.tile_pool(name="mask", bufs=3))
        spool = ctx.enter_context(tc.tile_pool(name="consts", bufs=1))
        epool = ctx.enter_context(tc.tile_pool(name="ew", bufs=3))
        rpool = ctx.enter_context(tc.tile_pool(name="recps", bufs=4, space="PSUM"))
        gpsum = ctx.enter_context(tc.tile_pool(name="gips", bufs=2, space="PSUM"))

        def load_w(h):
            t = wpool.tile([128, UT, GT, 128], F16, tag=h.name)
            nc.sync.dma_start(out=t[:], in_=h.ap())
            return t

        w0sb = load_w(w0_h)
        u0sb = load_w(u0_h)
        w1sb = load_w(w1_h)
        u1sb = load_w(u1_h)

        bigc = None
        if has_mask:
            bigc = spool.tile([1, 128], F16, tag="bigc")
            nc.vector.memset(bigc[:], BIG)

        hz0 = spool.tile([128, UT, BC], F16, tag="hz0")
        nc.vector.memset(hz0[:], 0.0)

        # --- pipeline stage builders (generators emitting one PE slice/step) -

        def load_x_chunk(c):
            xc = xpool.tile([128, UT, CH, BC], F16, tag="xc")
            nc.sync.dma_start(
                out=xc[:], in_=xT_h.ap()[:, :, c * CH:(c + 1) * CH, :]
            )
            return xc

        def load_mask_chunk(c):
            if not has_mask:
                return None
            mk = mpool.tile([1, CH, BC], F16, tag="mk")
            msl = msk_h.ap()[c * CH:(c + 1) * CH, :]
            nc.sync.dma_start(
                out=mk[:],
                in_=bass.AP(tensor=msl.tensor, offset=msl.offset,
                            ap=[[0, 1]] + list(msl.ap)),
            )
            return mk

        def gi_units(wsb, rhs_of_ut, mchunk, gi_tile):
            """Yield GT work-units; each computes gi[:, gt, :, :] (N = CH*BC)."""
            for gt in range(GT):
                ps = gpsum.tile([128, CH, BC], F32, tag="gips")
                for ut in range(UT):
                    nc.tensor.matmul(
                        ps[:], wsb[:, ut, gt, :], rhs_of_ut(ut),
                        start=(ut == 0), stop=(ut == UT - 1) and not
                        (has_mask and gt < UT),
                    )
                if has_mask and gt < UT:  # z gates: += BIG * (1 - m_t)
                    nc.tensor.matmul(
                        ps[:], bigc[:, :], mchunk[:, :, :],
                        start=False, stop=True,
                    )
                # PSUM f32 -> fp16 gi ring slice, on ScalarE (DVE is busier)
                nc.scalar.activation(out=gi_tile[:, gt, :, :], in_=ps[:],
                                     func=AF.Copy)
                yield

        def rec_step(usb, gi_tile, tt, h_prev, h_out):
            """One recurrence step: MM block + EW chain. h fp16 [128, UT, BC]."""
            ps = rpool.tile([128, GT, BC], F32, tag="recps")
            # z,r gate tiles first so the EW add can start while h-gate MMs run
            for gt in list(range(2 * UT)) + list(range(2 * UT, GT)):
                for ut in range(UT):
                    nc.tensor.matmul(
                        ps[:, gt, :], usb[:, ut, gt, :], h_prev[:, ut, :],
                        start=(ut == 0), stop=(ut == UT - 1),
                    )

            t1 = epool.tile([128, 2 * UT, BC], F32, tag="t1")
            nc.vector.tensor_add(t1[:], ps[:, 0:2 * UT, :],
                                 gi_tile[:, 0:2 * UT, tt, :])
            r_ = epool.tile([128, UT, BC], F32, tag="r")
            nc.scalar.activation(out=r_[:], in_=t1[:, UT:2 * UT, :],
                                 func=AF.Sigmoid)
            w_ = epool.tile([128, UT, BC], F32, tag="w")
            nc.scalar.activation(out=w_[:], in_=t1[:, 0:UT, :],
                                 func=AF.Sigmoid, scale=-1.0)
            hr = epool.tile([128, UT, BC], F32, tag="hr")
            nc.vector.tensor_mul(hr[:], r_[:], ps[:, 2 * UT:GT, :])
            cp = epool.tile([128, UT, BC], F32, tag="cp")
            nc.vector.tensor_add(cp[:], hr[:], gi_tile[:, 2 * UT:GT, tt, :])
            cd = epool.tile([128, UT, BC], F32, tag="cd")
            nc.scalar.activation(out=cd[:], in_=cp[:], func=AF.Tanh)
            d_ = epool.tile([128, UT, BC], F32, tag="d")
            nc.vector.tensor_sub(d_[:], cd[:], h_prev)
            e_ = epool.tile([128, UT, BC], F32, tag="e")
            nc.vector.tensor_mul(e_[:], w_[:], d_[:])
            nc.vector.tensor_add(h_out, h_prev, e_[:])

        # --- the pipeline ----------------------------------------------------

        h0_prev = hz0[:]
        h1_prev = hz0[:]
        gi0_cur = None        # gi0 ring tile for chunk c (being consumed by L0)
        gi1_cur = None        # gi1 ring tile for chunk c-1 (consumed by L1)
        h0_chunks = {}        # c -> h0 ring tile
        h1_cur = None

        def emit_gi0(c):
            xc = load_x_chunk(c)
            mk = load_mask_chunk(c)
            gi = gpool.tile([128, GT, CH, BC], F16, tag="gi0")
            return gi, gi_units(w0sb, lambda ut: xc[:, ut, :, :], mk, gi)

        def emit_gi1(c):
            mk = load_mask_chunk(c)
            hc = h0_chunks.pop(c)
            gi = gpool.tile([128, GT, CH, BC], F16, tag="gi1")
            return gi, gi_units(w1sb, lambda ut: hc[:, :, ut, :], mk, gi)

        def drain(unit_iter):
            if unit_iter is not None:
                for _ in unit_iter:
                    pass

        # prologue: gi0 for chunks 0; then for each chunk c: run L0(c) steps
        # interleaved with L1(c-1) steps and gi0(c+1)/gi1(c) unit slices.
        gi0_cur, it = emit_gi0(0)
        drain(it)

        for c in range(NCH):
            # gi0 for next chunk: interleave its units between steps below
            gi0_next, gi0_it = emit_gi0(c + 1) if c + 1 < NCH else (None, None)

            h0c = hpool.tile([128, CH, UT, BC], F16, tag="h0c")
            h0_chunks[c] = h0c
            if c > 0:
                h1c = hpool.tile([128, CH, UT, BC], F16, tag="h1c")

            # gi1(c-1) units must complete before L1(c-1) starts; emitted at
            # the top of this chunk so their PE work hides prior EW tails.
            if c > 0:
                gi1_cur, it = emit_gi1(c - 1)
                drain(it)

            for tt in range(CH):
                rec_step(u0sb, gi0_cur, tt, h0_prev, h0c[:, tt])
                h0_prev = h0c[:, tt]
                if gi0_it is not None:
                    next(gi0_it, None)
                if c > 0:
                    rec_step(u1sb, gi1_cur, tt, h1_prev, h1c[:, tt])
                    h1_prev = h1c[:, tt]
                    if gi0_it is not None and tt % 2 == 1:
                        next(gi0_it, None)

            if c > 0:
                nc.sync.dma_start(
                    out=out_h.ap()[:, (c - 1) * CH:c * CH, :, :], in_=h1c[:]
                )
            drain(gi0_it)
            gi0_cur = gi0_next

        # epilogue: L1 for the last chunk
        gi1_cur, it = emit_gi1(NCH - 1)
        drain(it)
        h1c = hpool.tile([128, CH, UT, BC], F16, tag="h1c")
        for tt in range(CH):
            rec_step(u1sb, gi1_cur, tt, h1_prev, h1c[:, tt])
            h1_prev = h1c[:, tt]
        nc.sync.dma_start(out=out_h.ap()[:, T - CH:T, :, :], in_=h1c[:])
        nc.sync.dma_start(out=h0f_h.ap(), in_=h0_prev)
        nc.sync.dma_start(out=h1f_h.ap(), in_=h1_prev)

    nc.compile()
    return nc


def _get_nc(has_mask: bool):
    key = has_mask
    if key not in _BUILD_CACHE:
        _BUILD_CACHE[key] = _build(has_mask)
    return _BUILD_CACHE[key]


def kernel(x, mask, W0, U0, b0, W1, U1, b1):
    x = np.asarray(x, np.float32)
    mask = np.asarray(mask)
    assert x.shape == (B, T, D) and mask.shape == (B, T)

    if np.any(np.asarray(b0)) or np.any(np.asarray(b1)):
        raise NotImplementedError("nonzero GRU biases not supported")

    has_mask = not bool(mask.all())
    nc = _get_nc(has_mask)

    def pack_w(w):
        return np.ascontiguousarray(
            np.asarray(w, np.float32).reshape(UT, 128, GT, 128)
            .transpose(1, 0, 2, 3)
        ).astype(np.float16)

    w0t, u0t, w1t, u1t = pack_w(W0), pack_w(U0), pack_w(W1), pack_w(U1)

    in_maps = []
    for c in range(N_CORES):
        xc = x[c * BC:(c + 1) * BC]                      # [BC, T, D]
        xT = np.ascontiguousarray(
            xc.reshape(BC, T, UT, 128).transpose(3, 2, 1, 0)
        ).astype(np.float16)                              # [128, UT, T, BC]
        m = {"xT": xT, "w0t": w0t, "u0t": u0t, "w1t": w1t, "u1t": u1t}
        if has_mask:
            mc = mask[c * BC:(c + 1) * BC].T              # [T, BC]
            m["maskn"] = (1.0 - mc.astype(np.float32)).astype(np.float16)
        in_maps.append(m)

    res = run_bass_kernel_spmd(
        nc, in_maps, core_ids=list(range(N_CORES)),
        trace=bool(os.environ.get("BASS_GRU_TRACE")),
    )
    global LAST_RESULTS
    LAST_RESULTS = res

    output = np.empty((B, T, U), np.float32)
    h0f = np.empty((B, U), np.float32)
    h1f = np.empty((B, U), np.float32)
    for c in range(N_CORES):
        r = res.results[c]
        output[c * BC:(c + 1) * BC] = (
            r["h1seq"].astype(np.float32).transpose(3, 1, 2, 0).reshape(BC, T, U)
        )
        h0f[c * BC:(c + 1) * BC] = (
            r["h0f"].astype(np.float32).transpose(2, 1, 0).reshape(BC, U)
        )
        h1f[c * BC:(c + 1) * BC] = (
            r["h1f"].astype(np.float32).transpose(2, 1, 0).reshape(BC, U)
        )
    return output, h0f, h1f


# revision 9
# speedup vs baseline: 1.8339x; 1.1720x over previous
"""Trainium2 Bass kernel for a 2-layer stacked GRU (Keras reset_after=True)
with sequence masking.

Problem: x [64, 512, 512], 2x GRU(512) with mask; returns (output [B,T,U],
h0_final [B,U], h1_final [B,U]).

Strategy: data-parallel over batch (8 cores x 8 sequences each). On each core
a single software pipeline over time-chunks of CH steps:

    GI0(c): W0-matmuls on x chunk c        -> gi0 SBUF ring   (PE + ACT copy)
    L0(c):  CH steps of layer-0 recurrence -> h0 chunk ring
    GI1(c): W1-matmuls on h0 chunk c       -> gi1 SBUF ring
    L1(c):  CH steps of layer-1 recurrence -> output chunk (DMA out)

L1 runs one chunk behind L0 and their per-step emissions are interleaved, so
each layer's serial elementwise chain hides under the other layer's matmul
block. GI0/GI1 matmul slices are spread between steps as extra PE filler.

The mask is folded into the z-gate pre-activation in GI (an extra K=1 matmul
adds +BIG*(1-m_t) to iz), which makes z ~= 1 (w ~= 0) on masked steps, so
h carries through unchanged - no per-step mask ops at all. The masked h1
state sequence equals the reference output exactly (out_t and h1_t follow
identical select recurrences, and layer-1's input on masked steps is never
observable in any output).

Per-step math (w = sigmoid(-(iz+hz)) = 1-z):
    h_next = h + w * (tanh(ih + r*hh) - h)

All matmul operands fp16 (10-bit mantissa; FWL-eligible like bf16), PSUM
accumulation fp32, h state carried in fp16, pre-activations assembled in f32.

Layouts per core (partition dim first):
  xT      [128, 4, T, 8]    fp16  (d%128, d//128, t, b)
  W/U     [128, 4, 12, 128] fp16  (k%128, k//128, gate_tile, gate%128)
  gi ring [128, 12, CH, 8]  fp16  (g%128, g//128, tt, b)
  h ring  [128, CH, 4, 8]   fp16  (u%128, tt, u//128, b)
"""

import os
import sys

for _p in ("/opt/trn_rl_repo",):
    if _p not in sys.path:
        sys.path.append(_p)

import numpy as np

import concourse.bass as bass
import concourse.tile as tile
from concourse import mybir, bacc
from concourse.bass_utils import run_bass_kernel_spmd

B, T, D, U = 64, 512, 512, 512
G = 3 * U
N_CORES = 8
BC = B // N_CORES      # 8 sequences per core
UT = U // 128          # 4 unit tiles
GT = G // 128          # 12 gate tiles
CH = 16                # time-chunk (pipeline granularity)
BIG = 30.0             # z-gate pre-activation offset on masked steps

F32 = mybir.dt.float32
F16 = mybir.dt.float16
AF = mybir.ActivationFunctionType
OP = mybir.AluOpType

_BUILD_CACHE = {}


def _build(has_mask: bool):
    nc = bacc.Bacc("TRN2", target_bir_lowering=False, debug=False,
                   num_devices=N_CORES)

    xT_h = nc.dram_tensor("xT", (128, UT, T, BC), F16, kind="ExternalInput")
    w0_h = nc.dram_tensor("w0t", (128, UT, GT, 128), F16, kind="ExternalInput")
    u0_h = nc.dram_tensor("u0t", (128, UT, GT, 128), F16, kind="ExternalInput")
    w1_h = nc.dram_tensor("w1t", (128, UT, GT, 128), F16, kind="ExternalInput")
    u1_h = nc.dram_tensor("u1t", (128, UT, GT, 128), F16, kind="ExternalInput")
    msk_h = None
    if has_mask:
        # (1 - mask) per (t, b), fp16
        msk_h = nc.dram_tensor("maskn", (T, BC), F16, kind="ExternalInput")

    out_h = nc.dram_tensor("h1seq", (128, T, UT, BC), F16, kind="ExternalOutput")
    h0f_h = nc.dram_tensor("h0f", (128, UT, BC), F16, kind="ExternalOutput")
    h1f_h = nc.dram_tensor("h1f", (128, UT, BC), F16, kind="ExternalOutput")

    NCH = T // CH

    from contextlib import ExitStack

    with tile.TileContext(nc) as tc, ExitStack() as ctx:
        wpool = ctx.enter_context(tc.tile_pool(name="weights", bufs=1))
        xpool = ctx.enter_context(tc.tile_pool(name="xchunk", bufs=3))
        gpool = ctx.enter_context(tc.tile_pool(name="girings", bufs=3))
        hpool = ctx.enter_context(tc.tile_pool(name="hrings", bufs=3))
        mpool = ctx.enter_context(tc.tile_pool(name="mask", bufs=3))
        spool = ctx.enter_context(tc.tile_pool(name="consts", bufs=1))
        epool = ctx.enter_context(tc.tile_pool(name="ew", bufs=3))
        rpool = ctx.enter_context(tc.tile_pool(name="recps", bufs=4, space="PSUM"))
        gpsum = ctx.enter_context(tc.tile_pool(name="gips", bufs=2, space="PSUM"))

        def load_w(h):
            t = wpool.tile([128, UT, GT, 128], F16, tag=h.name)
            nc.sync.dma_start(out=t[:], in_=h.ap())
            return t

        w0sb = load_w(w0_h)
        u0sb = load_w(u0_h)
        w1sb = load_w(w1_h)
        u1sb = load_w(u1_h)

        bigc = None
        if has_mask:
            bigc = spool.tile([1, 128], F16, tag="bigc")
            nc.vector.memset(bigc[:], BIG)

        hz0 = spool.tile([128, UT, BC], F16, tag="hz0")
        nc.vector.memset(hz0[:], 0.0)

        # --- pipeline stage builders (generators emitting one PE slice/step) -

        def load_x_chunk(c):
            xc = xpool.tile([128, UT, CH, BC], F16, tag="xc")
            nc.sync.dma_start(
                out=xc[:], in_=xT_h.ap()[:, :, c * CH:(c + 1) * CH, :]
            )
            return xc

        def load_mask_chunk(c):
            if not has_mask:
                return None
            mk = mpool.tile([1, CH, BC], F16, tag="mk")
            msl = msk_h.ap()[c * CH:(c + 1) * CH, :]
            nc.sync.dma_start(
                out=mk[:],
                in_=bass.AP(tensor=msl.tensor, offset=msl.offset,
                            ap=[[0, 1]] + list(msl.ap)),
            )
            return mk

        def gi_units(wsb, rhs_of_ut, mchunk, gi_tile):
            """Yield GT work-units; each computes gi[:, gt, :, :] (N = CH*BC)."""
            for gt in range(GT):
                ps = gpsum.tile([128, CH, BC], F32, tag="gips")
                for ut in range(UT):
                    nc.tensor.matmul(
                        ps[:], wsb[:, ut, gt, :], rhs_of_ut(ut),
                        start=(ut == 0), stop=(ut == UT - 1) and not
                        (has_mask and gt < UT),
                    )
                if has_mask and gt < UT:  # z gates: += BIG * (1 - m_t)
                    nc.tensor.matmul(
                        ps[:], bigc[:, :], mchunk[:, :, :],
                        start=False, stop=True,
                    )
                # PSUM f32 -> fp16 gi ring slice; alternate ACT/DVE to balance
                if gt % 2 == 0:
                    nc.scalar.activation(out=gi_tile[:, gt, :, :], in_=ps[:],
                                         func=AF.Copy)
                else:
                    nc.vector.tensor_copy(out=gi_tile[:, gt, :, :], in_=ps[:])
                yield

        def rec_step(usb, gi_tile, tt, h_prev, h_out):
            """One recurrence step: MM block + EW chain. h fp16 [128, UT, BC].

            PSUM deps are bank-granular, so r/z/h gate groups go to separate
            banks: the r-path (add+sigmoid) starts after 16 of 48 MMs and
            hides under the remaining groups. h' = (h - w*h) + w*cand, with
            (h - w*h) on GpSimd off the critical path.
            """
            psR = rpool.tile([128, UT, BC], F32, tag="psR", bufs=2)
            psZ = rpool.tile([128, UT, BC], F32, tag="psZ", bufs=2)
            psH = rpool.tile([128, UT, BC], F32, tag="psH", bufs=2)

            def mmgrp(ps, gt0):
                for j in range(UT):
                    for ut in range(UT):
                        nc.tensor.matmul(
                            ps[:, j, :], usb[:, ut, gt0 + j, :],
                            h_prev[:, ut, :],
                            start=(ut == 0), stop=(ut == UT - 1),
                        )

            mmgrp(psR, UT)          # r gates
            tR = epool.tile([128, UT, BC], F32, tag="tR")
            nc.vector.tensor_add(tR[:], psR[:], gi_tile[:, UT:2 * UT, tt, :])
            r_ = epool.tile([128, UT, BC], F32, tag="r")
            nc.scalar.activation(out=r_[:], in_=tR[:], func=AF.Sigmoid)

            mmgrp(psZ, 0)           # z gates
            tZ = epool.tile([128, UT, BC], F32, tag="tZ")
            nc.vector.tensor_add(tZ[:], psZ[:], gi_tile[:, 0:UT, tt, :])
            w_ = epool.tile([128, UT, BC], F32, tag="w")
            nc.scalar.activation(out=w_[:], in_=tZ[:], func=AF.Sigmoid,
                                 scale=-1.0)
            q_ = epool.tile([128, UT, BC], F32, tag="q")
            nc.gpsimd.tensor_mul(q_[:], w_[:], h_prev)
            s_ = epool.tile([128, UT, BC], F32, tag="s")
            nc.gpsimd.tensor_sub(s_[:], h_prev, q_[:])

            mmgrp(psH, 2 * UT)      # h gates
            hr = epool.tile([128, UT, BC], F32, tag="hr")
            nc.vector.tensor_mul(hr[:], r_[:], psH[:])
            cp = epool.tile([128, UT, BC], F32, tag="cp")
            nc.vector.tensor_add(cp[:], hr[:], gi_tile[:, 2 * UT:GT, tt, :])
            cd = epool.tile([128, UT, BC], F32, tag="cd")
            nc.scalar.activation(out=cd[:], in_=cp[:], func=AF.Tanh)
            m1 = epool.tile([128, UT, BC], F32, tag="m1")
            nc.vector.tensor_mul(m1[:], w_[:], cd[:])
            nc.vector.tensor_add(h_out, s_[:], m1[:])

        # --- the pipeline ----------------------------------------------------

        h0_prev = hz0[:]
        h1_prev = hz0[:]
        gi0_cur = None        # gi0 ring tile for chunk c (being consumed by L0)
        gi1_cur = None        # gi1 ring tile for chunk c-1 (consumed by L1)
        h0_chunks = {}        # c -> h0 ring tile
        h1_cur = None

        def emit_gi0(c):
            xc = load_x_chunk(c)
            mk = load_mask_chunk(c)
            gi = gpool.tile([128, GT, CH, BC], F16, tag="gi0")
            return gi, gi_units(w0sb, lambda ut: xc[:, ut, :, :], mk, gi)

        def emit_gi1(c):
            mk = load_mask_chunk(c)
            hc = h0_chunks.pop(c)
            gi = gpool.tile([128, GT, CH, BC], F16, tag="gi1")
            return gi, gi_units(w1sb, lambda ut: hc[:, :, ut, :], mk, gi)

        def drain(unit_iter):
            if unit_iter is not None:
                for _ in unit_iter:
                    pass

        # prologue: gi0 for chunks 0; then for each chunk c: run L0(c) steps
        # interleaved with L1(c-1) steps and gi0(c+1)/gi1(c) unit slices.
        gi0_cur, it = emit_gi0(0)
        drain(it)

        for c in range(NCH):
            # gi0 for next chunk: interleave its units between steps below
            gi0_next, gi0_it = emit_gi0(c + 1) if c + 1 < NCH else (None, None)

            h0c = hpool.tile([128, CH, UT, BC], F16, tag="h0c", bufs=4)
            h0_chunks[c] = h0c
            if c > 0:
                h1c = hpool.tile([128, CH, UT, BC], F16, tag="h1c", bufs=6)

            # gi1(c-1) units must complete before L1(c-1) starts; emitted at
            # the top of this chunk so their PE work hides prior EW tails.
            if c > 0:
                gi1_cur, it = emit_gi1(c - 1)
                drain(it)

            for tt in range(CH):
                rec_step(u0sb, gi0_cur, tt, h0_prev, h0c[:, tt])
                h0_prev = h0c[:, tt]
                if gi0_it is not None:
                    next(gi0_it, None)
                if c > 0:
                    rec_step(u1sb, gi1_cur, tt, h1_prev, h1c[:, tt])
                    h1_prev = h1c[:, tt]
                    if gi0_it is not None and tt % 2 == 1:
                        next(gi0_it, None)

            if c > 0:
                nc.gpsimd.dma_start(
                    out=out_h.ap()[:, (c - 1) * CH:c * CH, :, :], in_=h1c[:]
                )
            drain(gi0_it)
            gi0_cur = gi0_next

        # epilogue: L1 for the last chunk
        gi1_cur, it = emit_gi1(NCH - 1)
        drain(it)
        h1c = hpool.tile([128, CH, UT, BC], F16, tag="h1c", bufs=6)
        for tt in range(CH):
            rec_step(u1sb, gi1_cur, tt, h1_prev, h1c[:, tt])
            h1_prev = h1c[:, tt]
        nc.gpsimd.dma_start(out=out_h.ap()[:, T - CH:T, :, :], in_=h1c[:])
        nc.sync.dma_start(out=h0f_h.ap(), in_=h0_prev)
        nc.sync.dma_start(out=h1f_h.ap(), in_=h1_prev)

    nc.compile()
    return nc


def _get_nc(has_mask: bool):
    key = has_mask
    if key not in _BUILD_CACHE:
        _BUILD_CACHE[key] = _build(has_mask)
    return _BUILD_CACHE[key]


def kernel(x, mask, W0, U0, b0, W1, U1, b1):
    x = np.asarray(x, np.float32)
    mask = np.asarray(mask)
    assert x.shape == (B, T, D) and mask.shape == (B, T)

    if np.any(np.asarray(b0)) or np.any(np.asarray(b1)):
        raise NotImplementedError("nonzero GRU biases not supported")

    has_mask = not bool(mask.all())
    nc = _get_nc(has_mask)

    def pack_w(w):
        return np.ascontiguousarray(
            np.asarray(w, np.float32).reshape(UT, 128, GT, 128)
            .transpose(1, 0, 2, 3)
        ).astype(np.float16)

    w0t, u0t, w1t, u1t = pack_w(W0), pack_w(U0), pack_w(W1), pack_w(U1)

    in_maps = []
    for c in range(N_CORES):
        xc = x[c * BC:(c + 1) * BC]                      # [BC, T, D]
        xT = np.ascontiguousarray(
            xc.reshape(BC, T, UT, 128).transpose(3, 2, 1, 0)
        ).astype(np.float16)                              # [128, UT, T, BC]
        m = {"xT": xT, "w0t": w0t, "u0t": u0t, "w1t": w1t, "u1t": u1t}
        if has_mask:
            mc = mask[c * BC:(c + 1) * BC].T              # [T, BC]
            m["maskn"] = (1.0 - mc.astype(np.float32)).astype(np.float16)
        in_maps.append(m)

    res = run_bass_kernel_spmd(
        nc, in_maps, core_ids=list(range(N_CORES)),
        trace=bool(os.environ.get("BASS_GRU_TRACE")),
    )
    global LAST_RESULTS
    LAST_RESULTS = res

    output = np.empty((B, T, U), np.float32)
    h0f = np.empty((B, U), np.float32)
    h1f = np.empty((B, U), np.float32)
    for c in range(N_CORES):
        r = res.results[c]
        output[c * BC:(c + 1) * BC] = (
            r["h1seq"].astype(np.float32).transpose(3, 1, 2, 0).reshape(BC, T, U)
        )
        h0f[c * BC:(c + 1) * BC] = (
            r["h0f"].astype(np.float32).transpose(2, 1, 0).reshape(BC, U)
        )
        h1f[c * BC:(c + 1) * BC] = (
            r["h1f"].astype(np.float32).transpose(2, 1, 0).reshape(BC, U)
        )
    return output, h0f, h1f


# revision 13
# speedup vs baseline: 1.9742x; 1.0765x over previous
"""Trainium2 Bass kernel for a 2-layer stacked GRU (Keras reset_after=True)
with sequence masking.

Problem: x [64, 512, 512], 2x GRU(512) with mask; returns (output [B,T,U],
h0_final [B,U], h1_final [B,U]).

Strategy: data-parallel over batch (8 cores x 8 sequences each). On each core
a single software pipeline over time-chunks of CH steps:

    GI0(c): W0-matmuls on x chunk c        -> gi0 SBUF ring   (PE + ACT copy)
    L0(c):  CH steps of layer-0 recurrence -> h0 chunk ring
    GI1(c): W1-matmuls on h0 chunk c       -> gi1 SBUF ring
    L1(c):  CH steps of layer-1 recurrence -> output chunk (DMA out)

L1 runs one chunk behind L0 and their per-step emissions are interleaved, so
each layer's serial elementwise chain hides under the other layer's matmul
block. GI0/GI1 matmul slices are spread between steps as extra PE filler.

The mask is folded into the z-gate pre-activation in GI (an extra K=1 matmul
adds +BIG*(1-m_t) to iz), which makes z ~= 1 (w ~= 0) on masked steps, so
h carries through unchanged - no per-step mask ops at all. The masked h1
state sequence equals the reference output exactly (out_t and h1_t follow
identical select recurrences, and layer-1's input on masked steps is never
observable in any output).

Per-step math (w = sigmoid(-(iz+hz)) = 1-z):
    h_next = h + w * (tanh(ih + r*hh) - h)

All matmul operands fp16 (10-bit mantissa; FWL-eligible like bf16), PSUM
accumulation fp32, h state carried in fp16, pre-activations assembled in f32.

Layouts per core (partition dim first):
  xT      [128, 4, T, 8]    fp16  (d%128, d//128, t, b)
  W/U     [128, 4, 12, 128] fp16  (k%128, k//128, gate_tile, gate%128)
  gi ring [128, 12, CH, 8]  fp16  (g%128, g//128, tt, b)
  h ring  [128, CH, 4, 8]   fp16  (u%128, tt, u//128, b)
"""

import os
import sys

for _p in ("/opt/trn_rl_repo",):
    if _p not in sys.path:
        sys.path.append(_p)

import numpy as np

import concourse.bass as bass
import concourse.tile as tile
from concourse import mybir, bacc
from concourse.bass_utils import run_bass_kernel_spmd

B, T, D, U = 64, 512, 512, 512
G = 3 * U
N_CORES = 8
BC = B // N_CORES      # 8 sequences per core
UT = U // 128          # 4 unit tiles
GT = G // 128          # 12 gate tiles
CH = 32                # time-chunk (pipeline granularity)
BIG = 30.0             # z-gate pre-activation offset on masked steps

F32 = mybir.dt.float32
F16 = mybir.dt.float16
AF = mybir.ActivationFunctionType
OP = mybir.AluOpType

_BUILD_CACHE = {}


def _build(has_mask: bool):
    nc = bacc.Bacc("TRN2", target_bir_lowering=False, debug=False,
                   num_devices=N_CORES)

    xT_h = nc.dram_tensor("xT", (128, UT, T, BC), F16, kind="ExternalInput")
    w0_h = nc.dram_tensor("w0t", (128, UT, GT, 128), F16, kind="ExternalInput")
    u0_h = nc.dram_tensor("u0t", (128, UT, GT, 128), F16, kind="ExternalInput")
    w1_h = nc.dram_tensor("w1t", (128, UT, GT, 128), F16, kind="ExternalInput")
    u1_h = nc.dram_tensor("u1t", (128, UT, GT, 128), F16, kind="ExternalInput")
    msk_h = None
    if has_mask:
        # (1 - mask) per (t, b), fp16
        msk_h = nc.dram_tensor("maskn", (T, BC), F16, kind="ExternalInput")

    out_h = nc.dram_tensor("h1seq", (128, T, UT, BC), F16, kind="ExternalOutput")
    h0f_h = nc.dram_tensor("h0f", (128, UT, BC), F16, kind="ExternalOutput")
    h1f_h = nc.dram_tensor("h1f", (128, UT, BC), F16, kind="ExternalOutput")

    NCH = T // CH

    from contextlib import ExitStack

    with tile.TileContext(nc) as tc, ExitStack() as ctx:
        wpool = ctx.enter_context(tc.tile_pool(name="weights", bufs=1))
        xpool = ctx.enter_context(tc.tile_pool(name="xchunk", bufs=3))
        gpool = ctx.enter_context(tc.tile_pool(name="girings", bufs=3))
        hpool = ctx.enter_context(tc.tile_pool(name="hrings", bufs=3))
        mpool = ctx.enter_context(tc.tile_pool(name="mask", bufs=3))
        spool = ctx.enter_context(tc.tile_pool(name="consts", bufs=1))
        epool = ctx.enter_context(tc.tile_pool(name="ew", bufs=3))
        rpool = ctx.enter_context(tc.tile_pool(name="recps", bufs=4, space="PSUM"))
        gpsum = ctx.enter_context(tc.tile_pool(name="gips", bufs=2, space="PSUM"))

        def load_w(h):
            t = wpool.tile([128, UT, GT, 128], F16, tag=h.name)
            nc.sync.dma_start(out=t[:], in_=h.ap())
            return t

        w0sb = load_w(w0_h)
        u0sb = load_w(u0_h)
        w1sb = load_w(w1_h)
        u1sb = load_w(u1_h)

        bigc = None
        if has_mask:
            bigc = spool.tile([1, 128], F16, tag="bigc")
            nc.vector.memset(bigc[:], BIG)

        hz0 = spool.tile([128, UT, BC], F16, tag="hz0")
        nc.vector.memset(hz0[:], 0.0)

        # --- pipeline stage builders (generators emitting one PE slice/step) -

        def load_x_chunk(c):
            xc = xpool.tile([128, UT, CH, BC], F16, tag="xc")
            nc.sync.dma_start(
                out=xc[:], in_=xT_h.ap()[:, :, c * CH:(c + 1) * CH, :]
            )
            return xc

        def load_mask_chunk(c):
            if not has_mask:
                return None
            mk = mpool.tile([1, CH, BC], F16, tag="mk")
            msl = msk_h.ap()[c * CH:(c + 1) * CH, :]
            nc.sync.dma_start(
                out=mk[:],
                in_=bass.AP(tensor=msl.tensor, offset=msl.offset,
                            ap=[[0, 1]] + list(msl.ap)),
            )
            return mk

        def gi_units(wsb, rhs_of_ut, mchunk, gi_tile):
            """Yield GT work-units; each computes gi[:, gt, :, :] (N = CH*BC)."""
            for gt in range(GT):
                ps = gpsum.tile([128, CH, BC], F32, tag="gips")
                for ut in range(UT):
                    nc.tensor.matmul(
                        ps[:], wsb[:, ut, gt, :], rhs_of_ut(ut),
                        start=(ut == 0), stop=(ut == UT - 1) and not
                        (has_mask and gt < UT),
                    )
                if has_mask and gt < UT:  # z gates: += BIG * (1 - m_t)
                    nc.tensor.matmul(
                        ps[:], bigc[:, :], mchunk[:, :, :],
                        start=False, stop=True,
                    )
                # PSUM f32 -> fp16 gi ring slice; alternate ACT/DVE to balance
                if gt % 2 == 0:
                    nc.scalar.activation(out=gi_tile[:, gt, :, :], in_=ps[:],
                                         func=AF.Copy)
                else:
                    nc.vector.tensor_copy(out=gi_tile[:, gt, :, :], in_=ps[:])
                yield

        def rec_step_p1(usb, gi_tile, tt, h_prev, lt):
            """Step phase 1: MM block + gates up to the tanh input.

            PSUM deps are bank-granular, so r/z/h gate groups go to separate
            banks: the r-path (add+sigmoid) starts after 16 of 48 MMs and
            hides under the remaining groups. h' = (h - w*h) + w*cand, with
            (h - w*h) on GpSimd off the critical path. lt ("0"/"1") keys the
            tile tags so the two interleaved layers don't share slots.
            """
            psR = rpool.tile([128, UT, BC], F32, tag="psR", bufs=2)
            psZ = rpool.tile([128, UT, BC], F32, tag="psZ", bufs=2)
            psH = rpool.tile([128, UT, BC], F32, tag="psH", bufs=2)

            def mmgrp(ps, gt0):
                for j in range(UT):
                    for ut in range(UT):
                        nc.tensor.matmul(
                            ps[:, j, :], usb[:, ut, gt0 + j, :],
                            h_prev[:, ut, :],
                            start=(ut == 0), stop=(ut == UT - 1),
                        )

            mmgrp(psR, UT)          # r gates
            tR = epool.tile([128, UT, BC], F32, tag="tR" + lt)
            nc.vector.tensor_add(tR[:], psR[:], gi_tile[:, UT:2 * UT, tt, :])
            r_ = epool.tile([128, UT, BC], F32, tag="r" + lt)
            nc.scalar.activation(out=r_[:], in_=tR[:], func=AF.Sigmoid)

            mmgrp(psZ, 0)           # z gates
            tZ = epool.tile([128, UT, BC], F32, tag="tZ" + lt)
            nc.vector.tensor_add(tZ[:], psZ[:], gi_tile[:, 0:UT, tt, :])
            w_ = epool.tile([128, UT, BC], F32, tag="w" + lt)
            nc.scalar.activation(out=w_[:], in_=tZ[:], func=AF.Sigmoid,
                                 scale=-1.0)
            q_ = epool.tile([128, UT, BC], F32, tag="q" + lt)
            nc.gpsimd.tensor_mul(q_[:], w_[:], h_prev)
            s_ = epool.tile([128, UT, BC], F32, tag="s" + lt)
            nc.gpsimd.tensor_sub(s_[:], h_prev, q_[:])

            mmgrp(psH, 2 * UT)      # h gates
            hr = epool.tile([128, UT, BC], F32, tag="hr" + lt)
            nc.vector.tensor_mul(hr[:], r_[:], psH[:])
            cp = epool.tile([128, UT, BC], F32, tag="cp" + lt)
            nc.gpsimd.tensor_add(cp[:], hr[:], gi_tile[:, 2 * UT:GT, tt, :])
            return cp, w_, s_

        def rec_step_p2(st, h_out, lt):
            """Step phase 2: tanh + final update (emitted after the other
            layer's phase 1 so its ACT/DVE ops don't block that layer)."""
            cp, w_, s_ = st
            cd = epool.tile([128, UT, BC], F32, tag="cd" + lt)
            nc.scalar.activation(out=cd[:], in_=cp[:], func=AF.Tanh)
            m1 = epool.tile([128, UT, BC], F32, tag="m1" + lt)
            nc.vector.tensor_mul(m1[:], w_[:], cd[:])
            nc.vector.tensor_add(h_out, s_[:], m1[:])

        # --- the pipeline ----------------------------------------------------

        h0_prev = hz0[:]
        h1_prev = hz0[:]
        gi0_cur = None        # gi0 ring tile for chunk c (being consumed by L0)
        gi1_cur = None        # gi1 ring tile for chunk c-1 (consumed by L1)
        h0_chunks = {}        # c -> h0 ring tile
        h1_cur = None

        def emit_gi0(c):
            xc = load_x_chunk(c)
            mk = load_mask_chunk(c)
            gi = gpool.tile([128, GT, CH, BC], F16, tag="gi0")
            return gi, gi_units(w0sb, lambda ut: xc[:, ut, :, :], mk, gi)

        def emit_gi1(c):
            mk = load_mask_chunk(c)
            hc = h0_chunks.pop(c)
            gi = gpool.tile([128, GT, CH, BC], F16, tag="gi1")
            return gi, gi_units(w1sb, lambda ut: hc[:, :, ut, :], mk, gi)

        def drain(unit_iter):
            if unit_iter is not None:
                for _ in unit_iter:
                    pass

        # prologue: gi0 for chunks 0; then for each chunk c: run L0(c) steps
        # interleaved with L1(c-1) steps and gi0(c+1)/gi1(c) unit slices.
        gi0_cur, it = emit_gi0(0)
        drain(it)

        for c in range(NCH):
            # gi0 for next chunk: interleave its units between steps below
            gi0_next, gi0_it = emit_gi0(c + 1) if c + 1 < NCH else (None, None)

            h0c = hpool.tile([128, CH, UT, BC], F16, tag="h0c", bufs=4)
            h0_chunks[c] = h0c
            if c > 0:
                h1c = hpool.tile([128, CH, UT, BC], F16, tag="h1c", bufs=6)

            # gi1(c-1) units must complete before L1(c-1) starts; emitted at
            # the top of this chunk so their PE work hides prior EW tails.
            if c > 0:
                gi1_cur, it = emit_gi1(c - 1)
                drain(it)

            for tt in range(CH):
                st0 = rec_step_p1(u0sb, gi0_cur, tt, h0_prev, "0")
                if gi0_it is not None:
                    next(gi0_it, None)
                st1 = None
                if c > 0:
                    st1 = rec_step_p1(u1sb, gi1_cur, tt, h1_prev, "1")
                rec_step_p2(st0, h0c[:, tt], "0")
                h0_prev = h0c[:, tt]
                if c > 0:
                    rec_step_p2(st1, h1c[:, tt], "1")
                    h1_prev = h1c[:, tt]
                    if gi0_it is not None and tt % 2 == 1:
                        next(gi0_it, None)

            if c > 0:
                nc.gpsimd.dma_start(
                    out=out_h.ap()[:, (c - 1) * CH:c * CH, :, :], in_=h1c[:]
                )
            drain(gi0_it)
            gi0_cur = gi0_next

        # epilogue: L1 for the last chunk
        gi1_cur, it = emit_gi1(NCH - 1)
        drain(it)
        h1c = hpool.tile([128, CH, UT, BC], F16, tag="h1c", bufs=6)
        for tt in range(CH):
            st1 = rec_step_p1(u1sb, gi1_cur, tt, h1_prev, "1")
            rec_step_p2(st1, h1c[:, tt], "1")
            h1_prev = h1c[:, tt]
        nc.gpsimd.dma_start(out=out_h.ap()[:, T - CH:T, :, :], in_=h1c[:])
        nc.sync.dma_start(out=h0f_h.ap(), in_=h0_prev)
        nc.sync.dma_start(out=h1f_h.ap(), in_=h1_prev)

    nc.compile()
    return nc


def _get_nc(has_mask: bool):
    key = has_mask
    if key not in _BUILD_CACHE:
        _BUILD_CACHE[key] = _build(has_mask)
    return _BUILD_CACHE[key]


def kernel(x, mask, W0, U0, b0, W1, U1, b1):
    x = np.asarray(x, np.float32)
    mask = np.asarray(mask)
    assert x.shape == (B, T, D) and mask.shape == (B, T)

    if np.any(np.asarray(b0)) or np.any(np.asarray(b1)):
        raise NotImplementedError("nonzero GRU biases not supported")

    has_mask = not bool(mask.all())
    nc = _get_nc(has_mask)

    def pack_w(w):
        return np.ascontiguousarray(
            np.asarray(w, np.float32).reshape(UT, 128, GT, 128)
            .transpose(1, 0, 2, 3)
        ).astype(np.float16)

    w0t, u0t, w1t, u1t = pack_w(W0), pack_w(U0), pack_w(W1), pack_w(U1)

    in_maps = []
    for c in range(N_CORES):
        xc = x[c * BC:(c + 1) * BC]                      # [BC, T, D]
        xT = np.ascontiguousarray(
            xc.reshape(BC, T, UT, 128).transpose(3, 2, 1, 0)
        ).astype(np.float16)                              # [128, UT, T, BC]
        m = {"xT": xT, "w0t": w0t, "u0t": u0t, "w1t": w1t, "u1t": u1t}
        if has_mask:
            mc = mask[c * BC:(c + 1) * BC].T              # [T, BC]
            m["maskn"] = (1.0 - mc.astype(np.float32)).astype(np.float16)
        in_maps.append(m)

    res = run_bass_kernel_spmd(
        nc, in_maps, core_ids=list(range(N_CORES)),
        trace=bool(os.environ.get("BASS_GRU_TRACE")),
    )
    global LAST_RESULTS
    LAST_RESULTS = res

    output = np.empty((B, T, U), np.float32)
    h0f = np.empty((B, U), np.float32)
    h1f = np.empty((B, U), np.float32)
    for c in range(N_CORES):
        r = res.results[c]
        output[c * BC:(c + 1) * BC] = (
            r["h1seq"].astype(np.float32).transpose(3, 1, 2, 0).reshape(BC, T, U)
        )
        h0f[c * BC:(c + 1) * BC] = (
            r["h0f"].astype(np.float32).transpose(2, 1, 0).reshape(BC, U)
        )
        h1f[c * BC:(c + 1) * BC] = (
            r["h1f"].astype(np.float32).transpose(2, 1, 0).reshape(BC, U)
        )
    return output, h0f, h1f


# revision 16
# speedup vs baseline: 1.9746x; 1.0002x over previous
"""Trainium2 Bass kernel for a 2-layer stacked GRU (Keras reset_after=True)
with sequence masking.

Problem: x [64, 512, 512], 2x GRU(512) with mask; returns (output [B,T,U],
h0_final [B,U], h1_final [B,U]).

Strategy: data-parallel over batch (8 cores x 8 sequences each). On each core
a single software pipeline over time-chunks of CH steps:

    GI0(c): W0-matmuls on x chunk c        -> gi0 SBUF ring   (PE + ACT copy)
    L0(c):  CH steps of layer-0 recurrence -> h0 chunk ring
    GI1(c): W1-matmuls on h0 chunk c       -> gi1 SBUF ring
    L1(c):  CH steps of layer-1 recurrence -> output chunk (DMA out)

L1 runs one chunk behind L0 and their per-step emissions are interleaved, so
each layer's serial elementwise chain hides under the other layer's matmul
block. GI0/GI1 matmul slices are spread between steps as extra PE filler.

The mask is folded into the z-gate pre-activation in GI (an extra K=1 matmul
adds +BIG*(1-m_t) to iz), which makes z ~= 1 (w ~= 0) on masked steps, so
h carries through unchanged - no per-step mask ops at all. The masked h1
state sequence equals the reference output exactly (out_t and h1_t follow
identical select recurrences, and layer-1's input on masked steps is never
observable in any output).

Per-step math (w = sigmoid(-(iz+hz)) = 1-z):
    h_next = h + w * (tanh(ih + r*hh) - h)

All matmul operands fp16 (10-bit mantissa; FWL-eligible like bf16), PSUM
accumulation fp32, h state carried in fp16, pre-activations assembled in f32.

Layouts per core (partition dim first):
  xT      [128, 4, T, 8]    fp16  (d%128, d//128, t, b)
  W/U     [128, 4, 12, 128] fp16  (k%128, k//128, gate_tile, gate%128)
  gi ring [128, 12, CH, 8]  fp16  (g%128, g//128, tt, b)
  h ring  [128, CH, 4, 8]   fp16  (u%128, tt, u//128, b)
"""

import os
import sys

for _p in ("/opt/trn_rl_repo",):
    if _p not in sys.path:
        sys.path.append(_p)

import numpy as np

import concourse.bass as bass
import concourse.tile as tile
from concourse import mybir, bacc
from concourse.bass_utils import run_bass_kernel_spmd

B, T, D, U = 64, 512, 512, 512
G = 3 * U
N_CORES = 8
BC = B // N_CORES      # 8 sequences per core
UT = U // 128          # 4 unit tiles
GT = G // 128          # 12 gate tiles
CH = 32                # time-chunk (pipeline granularity)
BIG = 30.0             # z-gate pre-activation offset on masked steps

F32 = mybir.dt.float32
F16 = mybir.dt.float16
AF = mybir.ActivationFunctionType
OP = mybir.AluOpType

_BUILD_CACHE = {}


def _build(has_mask: bool):
    nc = bacc.Bacc("TRN2", target_bir_lowering=False, debug=False,
                   num_devices=N_CORES)

    xT_h = nc.dram_tensor("xT", (128, UT, T, BC), F16, kind="ExternalInput")
    w0_h = nc.dram_tensor("w0t", (128, UT, GT, 128), F16, kind="ExternalInput")
    u0_h = nc.dram_tensor("u0t", (128, UT, GT, 128), F16, kind="ExternalInput")
    w1_h = nc.dram_tensor("w1t", (128, UT, GT, 128), F16, kind="ExternalInput")
    u1_h = nc.dram_tensor("u1t", (128, UT, GT, 128), F16, kind="ExternalInput")
    msk_h = None
    if has_mask:
        # (1 - mask) per (t, b), fp16
        msk_h = nc.dram_tensor("maskn", (T, BC), F16, kind="ExternalInput")

    out_h = nc.dram_tensor("h1seq", (128, T, UT, BC), F16, kind="ExternalOutput")
    h0f_h = nc.dram_tensor("h0f", (128, UT, BC), F16, kind="ExternalOutput")
    h1f_h = nc.dram_tensor("h1f", (128, UT, BC), F16, kind="ExternalOutput")

    NCH = T // CH

    from contextlib import ExitStack

    with tile.TileContext(nc) as tc, ExitStack() as ctx:
        wpool = ctx.enter_context(tc.tile_pool(name="weights", bufs=1))
        xpool = ctx.enter_context(tc.tile_pool(name="xchunk", bufs=3))
        gpool = ctx.enter_context(tc.tile_pool(name="girings", bufs=3))
        hpool = ctx.enter_context(tc.tile_pool(name="hrings", bufs=3))
        mpool = ctx.enter_context(tc.tile_pool(name="mask", bufs=3))
        spool = ctx.enter_context(tc.tile_pool(name="consts", bufs=1))
        epool = ctx.enter_context(tc.tile_pool(name="ew", bufs=3))
        rpool = ctx.enter_context(tc.tile_pool(name="recps", bufs=4, space="PSUM"))
        gpsum = ctx.enter_context(tc.tile_pool(name="gips", bufs=2, space="PSUM"))

        def load_w(h):
            t = wpool.tile([128, UT, GT, 128], F16, tag=h.name)
            nc.sync.dma_start(out=t[:], in_=h.ap())
            return t

        w0sb = load_w(w0_h)
        u0sb = load_w(u0_h)
        w1sb = load_w(w1_h)
        u1sb = load_w(u1_h)

        bigc = None
        if has_mask:
            bigc = spool.tile([1, 128], F16, tag="bigc")
            nc.vector.memset(bigc[:], BIG)

        hz0 = spool.tile([128, UT, BC], F16, tag="hz0")
        nc.vector.memset(hz0[:], 0.0)

        # --- pipeline stage builders (generators emitting one PE slice/step) -

        def load_x_chunk(c):
            xc = xpool.tile([128, UT, CH, BC], F16, tag="xc")
            nc.sync.dma_start(
                out=xc[:], in_=xT_h.ap()[:, :, c * CH:(c + 1) * CH, :]
            )
            return xc

        def load_mask_chunk(c):
            if not has_mask:
                return None
            mk = mpool.tile([1, CH, BC], F16, tag="mk")
            msl = msk_h.ap()[c * CH:(c + 1) * CH, :]
            nc.sync.dma_start(
                out=mk[:],
                in_=bass.AP(tensor=msl.tensor, offset=msl.offset,
                            ap=[[0, 1]] + list(msl.ap)),
            )
            return mk

        def gi_units(wsb, rhs_of_ut, mchunk, gi_tile):
            """Yield GT work-units; each computes gi[:, gt, :, :] (N = CH*BC)."""
            for gt in range(GT):
                ps = gpsum.tile([128, CH, BC], F32, tag="gips")
                for ut in range(UT):
                    nc.tensor.matmul(
                        ps[:], wsb[:, ut, gt, :], rhs_of_ut(ut),
                        start=(ut == 0), stop=(ut == UT - 1) and not
                        (has_mask and gt < UT),
                    )
                if has_mask and gt < UT:  # z gates: += BIG * (1 - m_t)
                    nc.tensor.matmul(
                        ps[:], bigc[:, :], mchunk[:, :, :],
                        start=False, stop=True,
                    )
                # PSUM f32 -> fp16 gi ring slice; alternate ACT/DVE to balance
                if gt % 2 == 0:
                    nc.scalar.activation(out=gi_tile[:, gt, :, :], in_=ps[:],
                                         func=AF.Copy)
                else:
                    nc.vector.tensor_copy(out=gi_tile[:, gt, :, :], in_=ps[:])
                yield

        def rec_step_p1(usb, gi_tile, tt, h_prev, lt, defer_tail=False):
            """Step phase 1: MM block + gates up to the tanh input.

            PSUM deps are bank-granular, so r/z/h gate groups go to separate
            banks: the r-path (add+sigmoid) starts after 16 of 48 MMs and
            hides under the remaining groups. h' = (h - w*h) + w*cand, with
            (h - w*h) on GpSimd off the critical path. lt ("0"/"1") keys the
            tile tags so the two interleaved layers don't share slots.
            """
            psR = rpool.tile([128, UT, BC], F32, tag="psR", bufs=2)
            psZ = rpool.tile([128, UT, BC], F32, tag="psZ", bufs=2)
            psH = rpool.tile([128, UT, BC], F32, tag="psH", bufs=2)

            def mmgrp(ps, gt0):
                for j in range(UT):
                    for ut in range(UT):
                        nc.tensor.matmul(
                            ps[:, j, :], usb[:, ut, gt0 + j, :],
                            h_prev[:, ut, :],
                            start=(ut == 0), stop=(ut == UT - 1),
                        )

            mmgrp(psR, UT)          # r gates
            tR = epool.tile([128, UT, BC], F32, tag="tR" + lt)
            nc.vector.tensor_add(tR[:], psR[:], gi_tile[:, UT:2 * UT, tt, :])
            r_ = epool.tile([128, UT, BC], F32, tag="r" + lt)
            nc.scalar.activation(out=r_[:], in_=tR[:], func=AF.Sigmoid)

            mmgrp(psZ, 0)           # z gates
            tZ = epool.tile([128, UT, BC], F32, tag="tZ" + lt)
            nc.vector.tensor_add(tZ[:], psZ[:], gi_tile[:, 0:UT, tt, :])
            mmgrp(psH, 2 * UT)      # h gates

            st = {"tZ": tZ, "r": r_, "psH": psH, "gi": gi_tile, "tt": tt,
                  "h_prev": h_prev}
            if not defer_tail:
                _step_tail(st, lt)
            return st

        def _step_tail(st, lt):
            """w-sigmoid + GpSimd precompute + candidate assembly."""
            w_ = epool.tile([128, UT, BC], F32, tag="w" + lt)
            nc.scalar.activation(out=w_[:], in_=st["tZ"][:], func=AF.Sigmoid,
                                 scale=-1.0)
            q_ = epool.tile([128, UT, BC], F32, tag="q" + lt)
            nc.gpsimd.tensor_mul(q_[:], w_[:], st["h_prev"])
            s_ = epool.tile([128, UT, BC], F32, tag="s" + lt)
            nc.gpsimd.tensor_sub(s_[:], st["h_prev"], q_[:])
            hr = epool.tile([128, UT, BC], F32, tag="hr" + lt)
            nc.vector.tensor_mul(hr[:], st["r"][:], st["psH"][:])
            cp = epool.tile([128, UT, BC], F32, tag="cp" + lt)
            nc.gpsimd.tensor_add(cp[:], hr[:],
                                 st["gi"][:, 2 * UT:GT, st["tt"], :])
            st["w"], st["s"], st["cp"] = w_, s_, cp

        def rec_step_p2(st, h_out, lt):
            """Step phase 2: tanh + final update (emitted after the other
            layer's phase 1 so its ACT/DVE ops don't block that layer).
            For the deferred layer, the tail runs here too."""
            if "cp" not in st:
                _step_tail(st, lt)
            cd = epool.tile([128, UT, BC], F32, tag="cd" + lt)
            nc.scalar.activation(out=cd[:], in_=st["cp"][:], func=AF.Tanh)
            m1 = epool.tile([128, UT, BC], F32, tag="m1" + lt)
            nc.vector.tensor_mul(m1[:], st["w"][:], cd[:])
            nc.vector.tensor_add(h_out, st["s"][:], m1[:])

        # --- the pipeline ----------------------------------------------------

        h0_prev = hz0[:]
        h1_prev = hz0[:]
        gi0_cur = None        # gi0 ring tile for chunk c (being consumed by L0)
        gi1_cur = None        # gi1 ring tile for chunk c-1 (consumed by L1)
        h0_chunks = {}        # c -> h0 ring tile
        h1_cur = None

        def emit_gi0(c):
            xc = load_x_chunk(c)
            mk = load_mask_chunk(c)
            gi = gpool.tile([128, GT, CH, BC], F16, tag="gi0")
            return gi, gi_units(w0sb, lambda ut: xc[:, ut, :, :], mk, gi)

        def emit_gi1(c):
            mk = load_mask_chunk(c)
            hc = h0_chunks.pop(c)
            gi = gpool.tile([128, GT, CH, BC], F16, tag="gi1")
            return gi, gi_units(w1sb, lambda ut: hc[:, :, ut, :], mk, gi)

        def drain(unit_iter):
            if unit_iter is not None:
                for _ in unit_iter:
                    pass

        # prologue: gi0 for chunks 0; then for each chunk c: run L0(c) steps
        # interleaved with L1(c-1) steps and gi0(c+1)/gi1(c) unit slices.
        gi0_cur, it = emit_gi0(0)
        drain(it)

        for c in range(NCH):
            # gi0 for next chunk: interleave its units between steps below
            gi0_next, gi0_it = emit_gi0(c + 1) if c + 1 < NCH else (None, None)

            h0c = hpool.tile([128, CH, UT, BC], F16, tag="h0c", bufs=4)
            h0_chunks[c] = h0c
            if c > 0:
                h1c = hpool.tile([128, CH, UT, BC], F16, tag="h1c", bufs=6)

            # gi1(c-1) units must complete before L1(c-1) starts; emitted at
            # the top of this chunk so their PE work hides prior EW tails.
            if c > 0:
                gi1_cur, it = emit_gi1(c - 1)
                drain(it)

            for tt in range(CH):
                st0 = rec_step_p1(u0sb, gi0_cur, tt, h0_prev, "0")
                if gi0_it is not None:
                    next(gi0_it, None)
                st1 = None
                if c > 0:
                    st1 = rec_step_p1(u1sb, gi1_cur, tt, h1_prev, "1",
                                       defer_tail=True)
                rec_step_p2(st0, h0c[:, tt], "0")
                h0_prev = h0c[:, tt]
                if c > 0:
                    rec_step_p2(st1, h1c[:, tt], "1")
                    h1_prev = h1c[:, tt]
                    if gi0_it is not None and tt % 2 == 1:
                        next(gi0_it, None)

            if c > 0:
                nc.gpsimd.dma_start(
                    out=out_h.ap()[:, (c - 1) * CH:c * CH, :, :], in_=h1c[:]
                )
            drain(gi0_it)
            gi0_cur = gi0_next

        # epilogue: L1 for the last chunk
        gi1_cur, it = emit_gi1(NCH - 1)
        drain(it)
        h1c = hpool.tile([128, CH, UT, BC], F16, tag="h1c", bufs=6)
        for tt in range(CH):
            st1 = rec_step_p1(u1sb, gi1_cur, tt, h1_prev, "1")
            rec_step_p2(st1, h1c[:, tt], "1")
            h1_prev = h1c[:, tt]
        nc.gpsimd.dma_start(out=out_h.ap()[:, T - CH:T, :, :], in_=h1c[:])
        nc.sync.dma_start(out=h0f_h.ap(), in_=h0_prev)
        nc.sync.dma_start(out=h1f_h.ap(), in_=h1_prev)

    nc.compile()
    return nc


def _get_nc(has_mask: bool):
    key = has_mask
    if key not in _BUILD_CACHE:
        _BUILD_CACHE[key] = _build(has_mask)
    return _BUILD_CACHE[key]


def kernel(x, mask, W0, U0, b0, W1, U1, b1):
    x = np.asarray(x, np.float32)
    mask = np.asarray(mask)
    assert x.shape == (B, T, D) and mask.shape == (B, T)

    if np.any(np.asarray(b0)) or np.any(np.asarray(b1)):
        raise NotImplementedError("nonzero GRU biases not supported")

    has_mask = not bool(mask.all())
    nc = _get_nc(has_mask)

    def pack_w(w):
        return np.ascontiguousarray(
            np.asarray(w, np.float32).reshape(UT, 128, GT, 128)
            .transpose(1, 0, 2, 3)
        ).astype(np.float16)

    w0t, u0t, w1t, u1t = pack_w(W0), pack_w(U0), pack_w(W1), pack_w(U1)

    in_maps = []
    for c in range(N_CORES):
        xc = x[c * BC:(c + 1) * BC]                      # [BC, T, D]
        xT = np.ascontiguousarray(
            xc.reshape(BC, T, UT, 128).transpose(3, 2, 1, 0)
        ).astype(np.float16)                              # [128, UT, T, BC]
        m = {"xT": xT, "w0t": w0t, "u0t": u0t, "w1t": w1t, "u1t": u1t}
        if has_mask:
            mc = mask[c * BC:(c + 1) * BC].T              # [T, BC]
            m["maskn"] = (1.0 - mc.astype(np.float32)).astype(np.float16)
        in_maps.append(m)

    res = run_bass_kernel_spmd(
        nc, in_maps, core_ids=list(range(N_CORES)),
        trace=bool(os.environ.get("BASS_GRU_TRACE")),
    )
    global LAST_RESULTS
    LAST_RESULTS = res

    output = np.empty((B, T, U), np.float32)
    h0f = np.empty((B, U), np.float32)
    h1f = np.empty((B, U), np.float32)
    for c in range(N_CORES):
        r = res.results[c]
        output[c * BC:(c + 1) * BC] = (
            r["h1seq"].astype(np.float32).transpose(3, 1, 2, 0).reshape(BC, T, U)
        )
        h0f[c * BC:(c + 1) * BC] = (
            r["h0f"].astype(np.float32).transpose(2, 1, 0).reshape(BC, U)
        )
        h1f[c * BC:(c + 1) * BC] = (
            r["h1f"].astype(np.float32).transpose(2, 1, 0).reshape(BC, U)
        )
    return output, h0f, h1f
